# revision 28
# baseline (speedup 1.0000x reference)
"""Bass/Trainium2 kernel for nn_MultiHeadAttention (T5-style rel-bias causal MHA).

Sharding: 8 cores = 2 batches x 4 head-groups (4 heads of 64 dims each).
Each core: projects q/k/v for its 256 proj rows, runs causal attention with
the T5 relative bias folded in as either a PSUM band preload (near-diagonal
blocks) or a constant exp-bias (bucket-31-saturated blocks), and computes a
partial out-projection. Host sums the 4 partials per batch.

v3: bf16 end-to-end (PSUM f32), ragged causal trimming, per-half const-block
detection, qs-outer section order with the qs=0 out-projection interleaved
into the attention stream, softmax-reciprocal replication on gpsimd
partition_broadcast (no PSUM/PE cost), staged band prefetch that never
blocks the projection stages.
"""
import math
import sys

sys.path.insert(0, "/opt/trn_rl_repo")

import ml_dtypes
import numpy as np

from concourse import bacc
import concourse.mybir as mybir
import concourse.tile as tile
from concourse.bass_utils import run_bass_kernel_spmd

F32 = mybir.dt.float32
BF16 = mybir.dt.bfloat16
FP8 = mybir.dt.float8e4
Exp = mybir.ActivationFunctionType.Exp
MUL = mybir.AluOpType.mult
NP_BF16 = ml_dtypes.bfloat16
NP_FP8 = ml_dtypes.float8_e4m3

B, L, D = 2, 2048, 1024
H, HD = 16, 64
NUM_BUCKETS, MAX_DISTANCE = 32, 128
HPC = 4  # heads per core
MPC = HPC * HD  # 256 proj rows per core
N_CORES = 8
NEG = -60.0  # additive mask value (exp(-60+s) == 0 in practice)

last_results = None  # BassKernelResults of the most recent run (for profiling)
_cached = None


def _bucket(rp: np.ndarray) -> np.ndarray:
    """T5 relative position bucket, mirrors the reference exactly."""
    sign = (rp > 0).astype(np.int32)
    n = np.abs(rp)
    max_exact = NUM_BUCKETS // 2
    n_safe = np.maximum(n, 1).astype(np.float32)
    vil = max_exact + (
        np.log(n_safe / max_exact)
        / math.log(MAX_DISTANCE / max_exact)
        * (NUM_BUCKETS - max_exact)
    ).astype(np.int32)
    vil = np.minimum(vil, NUM_BUCKETS - 1)
    buckets = np.where(n < max_exact, n, vil) + sign * max_exact
    return np.clip(buckets, 0, NUM_BUCKETS - 1)


def _build():
    nc = bacc.Bacc(trn_type="TRN2")

    qT_in = nc.dram_tensor("qT_in", [D, L], BF16, kind="ExternalInput")
    kT_in = nc.dram_tensor("kT_in", [D, L], BF16, kind="ExternalInput")
    vT_in = nc.dram_tensor("vT_in", [D, L], BF16, kind="ExternalInput")
    wq_in = nc.dram_tensor("wq_in", [128, 8, MPC], BF16, kind="ExternalInput")
    wk_in = nc.dram_tensor("wk_in", [128, 8, MPC], BF16, kind="ExternalInput")
    wv_in = nc.dram_tensor("wv_in", [128, 8, MPC], BF16, kind="ExternalInput")
    wo_in = nc.dram_tensor("wo_in", [128, 2, D], BF16, kind="ExternalInput")
    bq_in = nc.dram_tensor("bq_in", [128, 2], F32, kind="ExternalInput")
    bk_in = nc.dram_tensor("bk_in", [128, 2], F32, kind="ExternalInput")
    band_in = nc.dram_tensor("band_in", [HPC, 128, 768], BF16, kind="ExternalInput")
    id_in = nc.dram_tensor("id_in", [128, 128], BF16, kind="ExternalInput")
    outT = nc.dram_tensor("outT", [D, L], BF16, kind="ExternalOutput")

    with tile.TileContext(nc) as tc:
        with (
            tc.tile_pool(name="res", bufs=1) as pr,
            tc.tile_pool(name="qkv", bufs=1) as pqkv,
            tc.tile_pool(name="bandp", bufs=1) as pbandp,
        ):
            # band for all 4 heads, resident the whole kernel; DMAs are
            # issued on the gpsimd ring, staggered between projection stage
            # loads so they never sit ahead of PE-critical bytes in the
            # (FIFO) DMA queues.
            band_t = [
                pbandp.tile([128, 768], BF16, name=f"band{hh}")
                for hh in range(HPC)
            ]
            wo = pqkv.tile([128, 2, D], BF16)

            bq = pr.tile([128, 2], F32)
            bk = pr.tile([128, 2], F32)
            ident = pr.tile([128, 128], BF16)
            # warm the ACT exp table early, off the critical path
            warm = pr.tile([1, 2], F32)
            nc.vector.memset(warm[:], 0.0)
            nc.scalar.activation(warm[:], warm[:], Exp)
            ones_v = pr.tile([1, HD], BF16)
            nc.vector.memset(ones_v[:], 1.0)
            ones_f = pr.tile([1, HD], F32)
            nc.vector.memset(ones_f[:], 1.0)

            qTz = []
            for hh in range(HPC):
                t = pqkv.tile([128, L], BF16, name=f"qtz{hh}")
                nc.vector.memset(t[:].bitcast(F32), 0.0)
                qTz.append(t)
            kTt = [
                pqkv.tile([128, L], BF16, name=f"kt{mm}") for mm in range(2)
            ]
            vxg = []
            for g in range(4):
                t = pqkv.tile([128, 4, HPC, HD + 1], BF16, name=f"vx{g}")
                nc.vector.memset(t[:, :, :, HD], 1.0)
                vxg.append(t)
            y_norm_qs = [
                pqkv.tile([128, 2, 1024], BF16, name=f"yn{qq}")
                for qq in range(2)
            ]

            # ---------------- projections ----------------
            with (
                tc.tile_pool(name="proj", bufs=1) as pp,
                tc.tile_pool(name="stg", bufs=12) as pstg,
                tc.tile_pool(name="ppsum", bufs=8, space="PSUM") as pps,
            ):
                dma_engs = [nc.sync, nc.scalar]
                wq = pp.tile([128, 8, MPC], BF16)
                # split so the kc=0 matmuls can start after ~256KB; the
                # rest rides behind the kc=1 stage
                nc.sync.dma_start(wq[:, 0:2, :], wq_in[:, 0:2, :])
                wk = pp.tile([128, 8, MPC], BF16)
                wv = pp.tile([128, 8, MPC], BF16)

                # q/k: transposed locals [m, l] = W_c @ x.T (+bias)
                for src_d, w_sb, b_sb, dst in (
                    (qT_in, wq, bq, None),
                    (kT_in, wk, bk, kTt),
                ):
                    if dst is not None:
                        nc.scalar.dma_start(wk[:], wk_in[:])
                        nc.scalar.dma_start(ident[:], id_in[:])
                    psums = [
                        pps.tile([128, 512], F32, tag="qk", name=f"qkp{i}")
                        for i in range(8)
                    ]
                    for kc in range(8):
                        stg = pstg.tile([128, L], BF16, tag="stage")
                        off = 1 if dst is None else 0
                        eng = dma_engs[(kc + off) % len(dma_engs)]
                        eng.dma_start(
                            stg[:], src_d[128 * kc : 128 * kc + 128, :]
                        )
                        if dst is None and kc == 1:
                            nc.sync.dma_start(wq[:, 2:8, :], wq_in[:, 2:8, :])
                        if dst is None and kc == 7:
                            # biases ride behind the q stages (the q psum
                            # evacuations read them)
                            nc.scalar.dma_start(bq[:], bq_in[:])
                            nc.scalar.dma_start(bk[:], bk_in[:])
                        for m in range(2):
                            for n in range(4):
                                nc.tensor.matmul(
                                    psums[m * 4 + n][:],
                                    w_sb[:, kc, 128 * m : 128 * m + 128],
                                    stg[:, 512 * n : 512 * n + 512],
                                    start=(kc == 0),
                                    stop=(kc == 7),
                                )
                    for m in range(2):
                        for n in range(4):
                            if dst is None:
                                for sub in range(2):
                                    pb = 64 * sub
                                    out_ap = qTz[2 * m + sub][
                                        pb : pb + 64,
                                        512 * n : 512 * n + 512,
                                    ]
                                    in_ap = psums[m * 4 + n][pb : pb + 64, :]
                                    bias_ap = b_sb[pb : pb + 64, m : m + 1]
                                    if (n + sub) % 2:
                                        nc.scalar.add(out_ap, in_ap, bias_ap)
                                    else:
                                        nc.vector.tensor_scalar_add(
                                            out_ap, in_ap, bias_ap
                                        )
                            else:
                                out_ap = kTt[m][:, 512 * n : 512 * n + 512]
                                in_ap = psums[m * 4 + n][:]
                                bias_ap = b_sb[:, m : m + 1]
                                if n % 2:
                                    nc.scalar.add(out_ap, in_ap, bias_ap)
                                else:
                                    nc.vector.tensor_scalar_add(
                                        out_ap, in_ap, bias_ap
                                    )

                # v: natural layout [l, m]; lhsT = staged vT chunks
                nc.scalar.dma_start(wv[:], wv_in[:])
                nc.sync.dma_start(band_t[0][:], band_in[0])
                stgv = []
                for kc in range(8):
                    s = pstg.tile([128, L], BF16, tag="stage")
                    eng = dma_engs[kc % len(dma_engs)]
                    eng.dma_start(s[:], vT_in[128 * kc : 128 * kc + 128, :])
                    stgv.append(s)
                    if kc == 1:
                        nc.scalar.dma_start(band_t[1][:], band_in[1])
                    if kc == 3:
                        nc.sync.dma_start(wo[:], wo_in[:])
                    if kc == 5:
                        nc.scalar.dma_start(band_t[2][:], band_in[2])
                nc.sync.dma_start(band_t[3][:], band_in[3])
                for grp in range(2):
                    psv = [
                        pps.tile([128, MPC], F32, tag="qk", name=f"vps{i}")
                        for i in range(8)
                    ]
                    for kc in range(8):
                        for i in range(8):
                            li = grp * 8 + i
                            nc.tensor.matmul(
                                psv[i][:],
                                stgv[kc][:, 128 * li : 128 * li + 128],
                                wv[:, kc, :],
                                start=(kc == 0),
                                stop=(kc == 7),
                            )
                    for i in range(8):
                        li = grp * 8 + i
                        v_dst = vxg[li // 4][:, li % 4, :, 0:HD]
                        v_src = psv[i][:].rearrange(
                            "p (h d) -> p h d", h=HPC
                        )
                        if i % 2:
                            nc.scalar.copy(v_dst, v_src)
                        else:
                            nc.vector.tensor_copy(v_dst, v_src)

            # ---------------- attention + out-projection ----------------
            # Scores use full K=128 contraction: lhsT carries BOTH heads of
            # the m-tile; the zero rows of qT_z kill the other head exactly.
            # K=128 keeps the PE activity monitor warm (K=64 never warms).
            # Causal trimming: boundary blocks (128*ki inside the col half)
            # restrict preload/score/exp/AV to the live cols [s:512); with
            # bf16 operands small-N matmuls stay at 1 cycle/row.
            # Sections run qs-outer so the qs=0 out-projection interleaves
            # into the attention stream (PE work that needs no ACT, letting
            # the exp backlog drain) and its output DMA overlaps qs=1.
            with (
                tc.tile_pool(name="es", bufs=8) as pes,
                tc.tile_pool(name="misc", bufs=3) as pmisc,
                tc.tile_pool(name="ost", bufs=4) as post,
                tc.tile_pool(name="spsum", bufs=2, space="PSUM") as psc,
                tc.tile_pool(name="ypsum", bufs=1, space="PSUM") as psy,
                tc.tile_pool(name="opsum", bufs=2, space="PSUM") as pso,
            ):
                def _emit_norm(item):
                    rrow, pb, mt, qsi = item
                    # gpsimd broadcast only works from base partition 0:
                    # fill all 128 partitions, use the [pb:pb+64] slice
                    prep_sb = pmisc.tile([128, 1024], BF16, tag="prep")
                    nc.gpsimd.partition_broadcast(prep_sb[:], rrow[:])
                    nc.vector.tensor_tensor(
                        y_norm_qs[qsi][pb : pb + 64, mt, :],
                        y_norm_qs[qsi][pb : pb + 64, mt, :],
                        prep_sb[pb : pb + 64, :],
                        MUL,
                    )

                def _emit_outproj(qsi, qhs=(0, 1)):
                    paired = len(qhs) == 2
                    for n in range(8):
                        ost2 = (
                            post.tile(
                                [128, 1024], BF16, tag="ost2", name="ost2"
                            )
                            if paired
                            else None
                        )
                        for qh in qhs:
                            qoff = 512 * qh
                            qi = 2 * qsi + qh
                            po = pso.tile([128, 512], F32, tag="out")
                            for c in range(2):
                                nc.tensor.matmul(
                                    po[:],
                                    wo[:, c, 128 * n : 128 * n + 128],
                                    y_norm_qs[qsi][:, c, qoff : qoff + 512],
                                    start=(c == 0),
                                    stop=(c == 1),
                                )
                            if paired:
                                dst = ost2[:, 512 * qh : 512 * qh + 512]
                            else:
                                dst = post.tile(
                                    [128, 512], BF16, tag="ost", name="ost"
                                )
                            if qsi == 1 and (n + qh) % 2 == 1:
                                nc.scalar.copy(dst, po[:])
                            else:
                                nc.vector.tensor_copy(dst, po[:])
                            if not paired:
                                [nc.sync, nc.scalar][n % 2].dma_start(
                                    outT[
                                        128 * n : 128 * n + 128,
                                        512 * qi : 512 * qi + 512,
                                    ],
                                    dst,
                                )
                        if paired:
                            # 2KB dst rows: one DMA per 128x1024 pair
                            [nc.sync, nc.scalar][n % 2].dma_start(
                                outT[
                                    128 * n : 128 * n + 128,
                                    1024 * qsi : 1024 * qsi + 1024,
                                ],
                                ost2[:],
                            )

                pending_norm = None
                for qs in range(2):
                    for h in range(HPC):
                        band = band_t[h]
                        mt = h // 2
                        q0 = 1024 * qs
                        n_live = 8 * (qs + 1)
                        live_half = [
                            min(4 * (2 * qs + j + 1), 16) for j in (0, 1)
                        ]
                        yT = psy.tile([HD + 1, 1024], F32, tag="yT")
                        last = (qs, h) == (1, HPC - 1)
                        if last:
                            # flush the pending norm now so its gpsimd
                            # broadcast can't wedge between this section's
                            # two split-chain broadcasts
                            _emit_norm(pending_norm)
                            pending_norm = None
                        pending = None
                        for ki in range(n_live):
                            # per-half spec: (j, s=trim start, const block?)
                            specs = []
                            for j in (0, 1):
                                if ki >= live_half[j]:
                                    continue
                                gq = q0 + 512 * j
                                s = max(0, 128 * ki - gq)
                                const_b = 128 * ki <= gq - 240
                                specs.append((j, s, const_b))
                            sp = psc.tile([128, 1024], F32, tag="score")
                            for j, s, const_b in specs:
                                gq = q0 + 512 * j
                                if not const_b:
                                    u0 = gq + s - 128 * ki
                                    nc.tensor.matmul(
                                        sp[:, 512 * j + s : 512 * j + 512],
                                        ident[:],
                                        band[:, u0 : u0 + 512 - s],
                                        start=True,
                                        stop=False,
                                    )
                                nc.tensor.matmul(
                                    sp[:, 512 * j + s : 512 * j + 512],
                                    kTt[mt][:, 128 * ki : 128 * ki + 128],
                                    qTz[h][:, gq + s : gq + 512],
                                    start=const_b,
                                    stop=True,
                                )
                            es = pes.tile([128, 1024], BF16, tag="es")
                            # bias is identically 0 (band carries the whole
                            # shifted rel-bias): one contiguous exp per ki
                            j0, s0, _ = specs[0]
                            lo = 512 * j0 + s0
                            hi = 512 * specs[-1][0] + 512
                            nc.scalar.activation(
                                es[:, lo:hi], sp[:, lo:hi], Exp
                            )
                            if pending is not None:
                                pes_t, pspecs, pki = pending
                                for j, s, _ in pspecs:
                                    nc.tensor.matmul(
                                        yT[:, 512 * j + s : 512 * j + 512],
                                        vxg[pki // 4][:, pki % 4, h, :],
                                        pes_t[:, 512 * j + s : 512 * j + 512],
                                        start=(pki == 0),
                                        stop=(pki == live_half[j] - 1),
                                    )
                                if False and pki == live_half[0] - 1:
                                    # j=0 half of the final section is done:
                                    # evacuate + normalize it now so the
                                    # out-projection can start immediately
                                    # after the last AV
                                    pbl = 64 * (h % 2)
                                    nc.vector.tensor_copy(
                                        y_norm_qs[qs][
                                            pbl : pbl + 64, h // 2, 0:512
                                        ],
                                        yT[0:HD, 0:512],
                                    )
                                    nc.vector.tensor_copy(
                                        dcp_l[:, 0:512],
                                        yT[HD : HD + 1, 0:512],
                                    )
                                    nc.sync.dma_start(
                                        dT_l[0:64, :], dcp_l[:, 0:512]
                                    )
                                    with nc.allow_low_precision(
                                        reason="softmax recip bf16"
                                    ):
                                        nc.vector.reciprocal(
                                            rT_l[0:64, :], dT_l[0:64, :]
                                        )
                                    nc.sync.dma_start(
                                        rrow_l[:, 0:512], rT_l[0:64, :]
                                    )
                                    nc.gpsimd.partition_broadcast(
                                        prep_l[:, 0:512], rrow_l[:, 0:512]
                                    )
                                    nc.vector.tensor_tensor(
                                        y_norm_qs[qs][
                                            pbl : pbl + 64, h // 2, 0:512
                                        ],
                                        y_norm_qs[qs][
                                            pbl : pbl + 64, h // 2, 0:512
                                        ],
                                        prep_l[pbl : pbl + 64, 0:512],
                                        MUL,
                                    )
                            pending = (es, specs, ki)
                        pes_t, pspecs, pki = pending
                        for j, s, _ in pspecs:
                            nc.tensor.matmul(
                                yT[:, 512 * j + s : 512 * j + 512],
                                vxg[pki // 4][:, pki % 4, h, :],
                                pes_t[:, 512 * j + s : 512 * j + 512],
                                start=(pki == 0),
                                stop=(pki == live_half[j] - 1),
                            )
                        pb = 64 * (h % 2)
                        if not last:
                            # yT evac FIRST (releases the single yT buffer
                            # for the next section ASAP), then the recip
                            # chain; the replication + in-place multiply for
                            # the PREVIOUS section is emitted now (its rrow
                            # is long ready), so nothing stalls on it.
                            nc.vector.tensor_copy(
                                y_norm_qs[qs][pb : pb + 64, mt, :],
                                yT[0:HD, :],
                            )
                            dcp = pmisc.tile([1, 1024], F32, tag="dcp")
                            nc.vector.tensor_copy(
                                dcp[:], yT[HD : HD + 1, :]
                            )
                            dT = pmisc.tile([128, 8], F32, tag="dT")
                            nc.sync.dma_start(dT[:], dcp[:])
                            rT = pmisc.tile([128, 8], BF16, tag="rT")
                            with nc.allow_low_precision(
                                reason="softmax recip bf16"
                            ):
                                nc.vector.reciprocal(rT[:], dT[:])
                            rrow = pmisc.tile([1, 1024], BF16, tag="rrow")
                            nc.sync.dma_start(rrow[:], rT[:])
                            if pending_norm is not None:
                                _emit_norm(pending_norm)
                            pending_norm = (rrow, pb, mt, qs)
                        else:
                            # final section: lean chain — one custom-DVE
                            # reciprocal straight off the PSUM denominator
                            # row, then fp32 PE replication (psum from the
                            # outproj pool). Fewer semaphore hops than the
                            # reshape/broadcast path.
                            dcp = pmisc.tile([1, 1024], F32, tag="dcp")
                            nc.vector.tensor_copy(
                                dcp[:], yT[HD : HD + 1, :]
                            )
                            rrec = pmisc.tile([1, 1024], F32, tag="rrec")
                            nc.vector.reciprocal_approx_fast(rrec[:], dcp[:])
                            nc.vector.tensor_copy(
                                y_norm_qs[qs][pb : pb + 64, mt, :],
                                yT[0:HD, :],
                            )
                            _emit_outproj(0, (1,))
                            for half in range(2):
                                hof = 512 * half
                                rep = pso.tile([64, 512], F32, tag="out")
                                nc.tensor.matmul(
                                    rep[:],
                                    ones_f[:],
                                    rrec[:, hof : hof + 512],
                                    start=True,
                                    stop=True,
                                )
                                nc.vector.tensor_tensor(
                                    y_norm_qs[qs][
                                        pb : pb + 64, mt, hof : hof + 512
                                    ],
                                    y_norm_qs[qs][
                                        pb : pb + 64, mt, hof : hof + 512
                                    ],
                                    rep[:],
                                    MUL,
                                )
                        if (qs, h) == (1, 0):
                            # norm for (3, qs=0) just got emitted: y_norm[0]
                            # is complete; its out-projection now overlaps
                            # the remaining qs=1 attention sections. qh=1
                            # stays back to fill the final-chain gap.
                            _emit_outproj(0, (0,))
                _emit_outproj(1)

    nc.finalize()
    return nc


def _host_tables(rel_emb: np.ndarray):
    """Per-core-group band tables; rel_emb is [NUM_BUCKETS, H]."""
    d = np.arange(4095)
    rp = d - 2047  # key - query
    buckets = _bucket(rp)
    bands = []
    c31s = []
    for h in range(H):
        # softmax is shift-invariant per row; subtracting the saturated
        # bucket-31 bias from the whole row makes the far-block exp bias
        # exactly 0 (no per-half bias bookkeeping on the ACT engine)
        c31 = np.float32(rel_emb[31, h])
        vals = rel_emb[buckets, h].astype(np.float32) - c31
        vals = np.where(rp > 0, np.float32(NEG), vals)  # causal mask
        band_pad = np.full(4223, NEG, np.float32)
        band_pad[:4095] = vals
        # BS[r, u] = band_pad[4095 + r - (2048 + u)]; reads only ever hit
        # x in [2048, 2816) of the full table
        idx = 2047 + np.arange(128)[:, None] - np.arange(768)[None, :]
        bands.append(band_pad[idx].astype(NP_BF16))
    return bands


def _numpy_ref(query, key, value, attn_mask, key_padding_mask,
               Wq, bq, Wk, bk, Wv, bv, Wo, bo, rel_emb):
    """Exact numpy fallback for unexpected mask patterns."""
    q = (query @ Wq.T + bq).reshape(B, L, H, HD).transpose(0, 2, 1, 3)
    k = (key @ Wk.T + bk).reshape(B, L, H, HD).transpose(0, 2, 1, 3)
    v = (value @ Wv.T + bv).reshape(B, L, H, HD).transpose(0, 2, 1, 3)
    scores = np.einsum("bhqd,bhkd->bhqk", q, k) / math.sqrt(HD)
    rp = np.arange(L, dtype=np.int64)[None, :] - np.arange(L, dtype=np.int64)[:, None]
    rel = rel_emb[_bucket(rp)].transpose(2, 0, 1)
    scores = scores + rel[None]
    scores = np.where(attn_mask[None, None], scores, -np.inf)
    scores = np.where(key_padding_mask[:, None, None, :], scores, -np.inf)
    scores = scores - scores.max(-1, keepdims=True)
    e = np.exp(scores)
    attn = e / e.sum(-1, keepdims=True)
    out = np.einsum("bhqk,bhkd->bhqd", attn, v)
    out = out.transpose(0, 2, 1, 3).reshape(B, L, D)
    return (out @ Wo.T + bo).astype(np.float32)


def kernel(**inputs) -> np.ndarray:
    global _cached, last_results
    inp = {k: np.asarray(v) for k, v in inputs.items()}
    query, key, value = inp["query"], inp["key"], inp["value"]
    attn_mask, kpm = inp["attn_mask"], inp["key_padding_mask"]
    Wq, bq, Wk, bk = inp["Wq"], inp["bq"], inp["Wk"], inp["bk"]
    Wv, bv, Wo, bo = inp["Wv"], inp["bv"], inp["Wo"], inp["bo"]
    rel_emb = inp["rel_emb"]

    causal = np.array_equal(attn_mask, np.tril(np.ones((L, L), bool)))
    if not (causal and kpm.all()):
        return _numpy_ref(**inp)

    if _cached is None:
        _cached = _build()
    nc = _cached

    bands = _host_tables(rel_emb)
    ident = np.eye(128, dtype=NP_BF16)

    def _rearr_w(w_slice):  # [MPC, D] row-major weights -> [128, 8, MPC]
        arr = np.ascontiguousarray(w_slice.T)  # [D, MPC]
        return arr.reshape(8, 128, MPC).transpose(1, 0, 2).astype(NP_BF16)

    def _rearr_w8(w_slice):
        arr = np.ascontiguousarray(w_slice.T)
        return arr.reshape(8, 128, MPC).transpose(1, 0, 2).astype(NP_FP8)

    in_maps = []
    for c in range(N_CORES):
        b, hg = c // HPC, c % HPC
        rows = slice(MPC * hg, MPC * hg + MPC)
        heads = range(HPC * hg, HPC * hg + HPC)
        wo_c = np.ascontiguousarray(Wo[:, rows].T)  # [MPC, D]
        in_maps.append({
            "qT_in": query[b].T.astype(NP_BF16),
            "kT_in": key[b].T.astype(NP_BF16),
            "vT_in": value[b].T.astype(NP_BF16),
            "wq_in": _rearr_w(Wq[rows] / math.sqrt(HD)),
            "wk_in": _rearr_w(Wk[rows]),
            "wv_in": _rearr_w(Wv[rows]),
            "wo_in": wo_c.reshape(2, 128, D).transpose(1, 0, 2).astype(NP_BF16),
            "bq_in": np.ascontiguousarray(
                (bq[rows] / math.sqrt(HD)).reshape(2, 128).T.astype(np.float32)
            ),
            "bk_in": np.ascontiguousarray(
                bk[rows].reshape(2, 128).T.astype(np.float32)
            ),
            "band_in": np.stack([bands[h] for h in heads]),
            "id_in": ident,
        })

    res = run_bass_kernel_spmd(nc, in_maps, list(range(N_CORES)))
    last_results = res

    bo_eff = (
        bo.astype(np.float64) + bv.astype(np.float64) @ Wo.T.astype(np.float64)
    )
    out = np.empty((B, L, D), np.float32)
    for b in range(B):
        acc = np.zeros((D, L), np.float64)
        for hg in range(HPC):
            acc += res.results[b * HPC + hg]["outT"].astype(np.float64)
        out[b] = (acc.T + bo_eff[None, :]).astype(np.float32)
    return out


# revision 29
# speedup vs baseline: 1.0294x; 1.0294x over previous
"""Bass/Trainium2 kernel for nn_MultiHeadAttention (T5-style rel-bias causal MHA).

Sharding: 8 cores = 2 batches x 4 head-groups (4 heads of 64 dims each).
Each core: projects q/k/v for its 256 proj rows, runs causal attention with
the T5 relative bias folded in as either a PSUM band preload (near-diagonal
blocks) or a constant exp-bias (bucket-31-saturated blocks), and computes a
partial out-projection. Host sums the 4 partials per batch.

v3: bf16 end-to-end (PSUM f32), ragged causal trimming, per-half const-block
detection, qs-outer section order with the qs=0 out-projection interleaved
into the attention stream, softmax-reciprocal replication on gpsimd
partition_broadcast (no PSUM/PE cost), staged band prefetch that never
blocks the projection stages.
"""
import math
import sys

sys.path.insert(0, "/opt/trn_rl_repo")

import ml_dtypes
import numpy as np

from concourse import bacc
import concourse.mybir as mybir
import concourse.tile as tile
from concourse.bass_utils import run_bass_kernel_spmd

F32 = mybir.dt.float32
BF16 = mybir.dt.bfloat16
FP8 = mybir.dt.float8e4
Exp = mybir.ActivationFunctionType.Exp
MUL = mybir.AluOpType.mult
NP_BF16 = ml_dtypes.bfloat16
NP_FP8 = ml_dtypes.float8_e4m3

B, L, D = 2, 2048, 1024
H, HD = 16, 64
NUM_BUCKETS, MAX_DISTANCE = 32, 128
HPC = 4  # heads per core
MPC = HPC * HD  # 256 proj rows per core
N_CORES = 8
NEG = -60.0  # additive mask value (exp(-60+s) == 0 in practice)

last_results = None  # BassKernelResults of the most recent run (for profiling)
_cached = None


def _bucket(rp: np.ndarray) -> np.ndarray:
    """T5 relative position bucket, mirrors the reference exactly."""
    sign = (rp > 0).astype(np.int32)
    n = np.abs(rp)
    max_exact = NUM_BUCKETS // 2
    n_safe = np.maximum(n, 1).astype(np.float32)
    vil = max_exact + (
        np.log(n_safe / max_exact)
        / math.log(MAX_DISTANCE / max_exact)
        * (NUM_BUCKETS - max_exact)
    ).astype(np.int32)
    vil = np.minimum(vil, NUM_BUCKETS - 1)
    buckets = np.where(n < max_exact, n, vil) + sign * max_exact
    return np.clip(buckets, 0, NUM_BUCKETS - 1)


def _build():
    nc = bacc.Bacc(trn_type="TRN2")

    qT_in = nc.dram_tensor("qT_in", [D, L], BF16, kind="ExternalInput")
    kT_in = nc.dram_tensor("kT_in", [D, L], BF16, kind="ExternalInput")
    vT_in = nc.dram_tensor("vT_in", [D, L], BF16, kind="ExternalInput")
    wq_in = nc.dram_tensor("wq_in", [128, 8, MPC], BF16, kind="ExternalInput")
    wk_in = nc.dram_tensor("wk_in", [128, 8, MPC], BF16, kind="ExternalInput")
    wv_in = nc.dram_tensor("wv_in", [128, 8, MPC], BF16, kind="ExternalInput")
    wo_in = nc.dram_tensor("wo_in", [128, 2, D], BF16, kind="ExternalInput")
    bq_in = nc.dram_tensor("bq_in", [128, 2], F32, kind="ExternalInput")
    bk_in = nc.dram_tensor("bk_in", [128, 2], F32, kind="ExternalInput")
    band_in = nc.dram_tensor("band_in", [HPC, 128, 768], BF16, kind="ExternalInput")
    id_in = nc.dram_tensor("id_in", [128, 128], BF16, kind="ExternalInput")
    outT = nc.dram_tensor("outT", [D, L], BF16, kind="ExternalOutput")

    with tile.TileContext(nc) as tc:
        with (
            tc.tile_pool(name="res", bufs=1) as pr,
            tc.tile_pool(name="qkv", bufs=1) as pqkv,
            tc.tile_pool(name="bandp", bufs=1) as pbandp,
        ):
            # band for all 4 heads, resident the whole kernel; DMAs are
            # issued on the gpsimd ring, staggered between projection stage
            # loads so they never sit ahead of PE-critical bytes in the
            # (FIFO) DMA queues.
            band_t = [
                pbandp.tile([128, 768], BF16, name=f"band{hh}")
                for hh in range(HPC)
            ]
            wo = pqkv.tile([128, 2, D], BF16)

            bq = pr.tile([128, 2], F32)
            bk = pr.tile([128, 2], F32)
            ident = pr.tile([128, 128], BF16)
            # warm the ACT exp table early, off the critical path
            warm = pr.tile([1, 2], F32)
            nc.vector.memset(warm[:], 0.0)
            nc.scalar.activation(warm[:], warm[:], Exp)
            ones_v = pr.tile([1, HD], BF16)
            nc.vector.memset(ones_v[:], 1.0)
            ones_f = pr.tile([1, HD], F32)
            nc.vector.memset(ones_f[:], 1.0)

            qTz = []
            for hh in range(HPC):
                t = pqkv.tile([128, L], BF16, name=f"qtz{hh}")
                nc.vector.memset(t[:].bitcast(F32), 0.0)
                qTz.append(t)
            kTt = [
                pqkv.tile([128, L], BF16, name=f"kt{mm}") for mm in range(2)
            ]
            vxg = []
            for g in range(4):
                t = pqkv.tile([128, 4, HPC, HD + 1], BF16, name=f"vx{g}")
                nc.vector.memset(t[:, :, :, HD], 1.0)
                vxg.append(t)
            y_norm_qs = [
                pqkv.tile([128, 2, 1024], BF16, name=f"yn{qq}")
                for qq in range(2)
            ]

            # ---------------- projections ----------------
            with (
                tc.tile_pool(name="proj", bufs=1) as pp,
                tc.tile_pool(name="stg", bufs=12) as pstg,
                tc.tile_pool(name="ppsum", bufs=8, space="PSUM") as pps,
            ):
                dma_engs = [nc.sync, nc.scalar]
                wq = pp.tile([128, 8, MPC], BF16)
                # split so the kc=0 matmuls can start after ~256KB; the
                # rest rides behind the kc=1 stage
                nc.sync.dma_start(wq[:, 0:2, :], wq_in[:, 0:2, :])
                wk = pp.tile([128, 8, MPC], BF16)
                wv = pp.tile([128, 8, MPC], BF16)

                # q/k: transposed locals [m, l] = W_c @ x.T (+bias)
                for src_d, w_sb, b_sb, dst in (
                    (qT_in, wq, bq, None),
                    (kT_in, wk, bk, kTt),
                ):
                    if dst is not None:
                        nc.scalar.dma_start(wk[:], wk_in[:])
                        nc.scalar.dma_start(ident[:], id_in[:])
                    psums = [
                        pps.tile([128, 512], F32, tag="qk", name=f"qkp{i}")
                        for i in range(8)
                    ]
                    for kc in range(8):
                        stg = pstg.tile([128, L], BF16, tag="stage")
                        off = 1 if dst is None else 0
                        eng = dma_engs[(kc + off) % len(dma_engs)]
                        eng.dma_start(
                            stg[:], src_d[128 * kc : 128 * kc + 128, :]
                        )
                        if dst is None and kc == 1:
                            nc.sync.dma_start(wq[:, 2:8, :], wq_in[:, 2:8, :])
                        if dst is None and kc == 7:
                            # biases ride behind the q stages (the q psum
                            # evacuations read them)
                            nc.scalar.dma_start(bq[:], bq_in[:])
                            nc.scalar.dma_start(bk[:], bk_in[:])
                        for m in range(2):
                            for n in range(4):
                                nc.tensor.matmul(
                                    psums[m * 4 + n][:],
                                    w_sb[:, kc, 128 * m : 128 * m + 128],
                                    stg[:, 512 * n : 512 * n + 512],
                                    start=(kc == 0),
                                    stop=(kc == 7),
                                )
                    for m in range(2):
                        for n in range(4):
                            if dst is None:
                                for sub in range(2):
                                    pb = 64 * sub
                                    nc.vector.tensor_scalar_add(
                                        qTz[2 * m + sub][
                                            pb : pb + 64,
                                            512 * n : 512 * n + 512,
                                        ],
                                        psums[m * 4 + n][pb : pb + 64, :],
                                        b_sb[pb : pb + 64, m : m + 1],
                                    )
                            else:
                                nc.vector.tensor_scalar_add(
                                    kTt[m][:, 512 * n : 512 * n + 512],
                                    psums[m * 4 + n][:],
                                    b_sb[:, m : m + 1],
                                )

                # v: natural layout [l, m]; lhsT = staged vT chunks
                nc.scalar.dma_start(wv[:], wv_in[:])
                nc.sync.dma_start(band_t[0][:], band_in[0])
                stgv = []
                for kc in range(8):
                    s = pstg.tile([128, L], BF16, tag="stage")
                    eng = dma_engs[kc % len(dma_engs)]
                    eng.dma_start(s[:], vT_in[128 * kc : 128 * kc + 128, :])
                    stgv.append(s)
                    if kc == 1:
                        nc.scalar.dma_start(band_t[1][:], band_in[1])
                    if kc == 3:
                        nc.sync.dma_start(wo[:], wo_in[:])
                    if kc == 5:
                        nc.scalar.dma_start(band_t[2][:], band_in[2])
                nc.sync.dma_start(band_t[3][:], band_in[3])
                for grp in range(2):
                    psv = [
                        pps.tile([128, MPC], F32, tag="qk", name=f"vps{i}")
                        for i in range(8)
                    ]
                    for kc in range(8):
                        for i in range(8):
                            li = grp * 8 + i
                            nc.tensor.matmul(
                                psv[i][:],
                                stgv[kc][:, 128 * li : 128 * li + 128],
                                wv[:, kc, :],
                                start=(kc == 0),
                                stop=(kc == 7),
                            )
                    for i in range(8):
                        li = grp * 8 + i
                        nc.vector.tensor_copy(
                            vxg[li // 4][:, li % 4, :, 0:HD],
                            psv[i][:].rearrange("p (h d) -> p h d", h=HPC),
                        )

            # ---------------- attention + out-projection ----------------
            # Scores use full K=128 contraction: lhsT carries BOTH heads of
            # the m-tile; the zero rows of qT_z kill the other head exactly.
            # K=128 keeps the PE activity monitor warm (K=64 never warms).
            # Causal trimming: boundary blocks (128*ki inside the col half)
            # restrict preload/score/exp/AV to the live cols [s:512); with
            # bf16 operands small-N matmuls stay at 1 cycle/row.
            # Sections run qs-outer so the qs=0 out-projection interleaves
            # into the attention stream (PE work that needs no ACT, letting
            # the exp backlog drain) and its output DMA overlaps qs=1.
            with (
                tc.tile_pool(name="es", bufs=8) as pes,
                tc.tile_pool(name="misc", bufs=3) as pmisc,
                tc.tile_pool(name="ost", bufs=4) as post,
                tc.tile_pool(name="spsum", bufs=2, space="PSUM") as psc,
                tc.tile_pool(name="ypsum", bufs=1, space="PSUM") as psy,
                tc.tile_pool(name="opsum", bufs=2, space="PSUM") as pso,
            ):
                def _emit_norm(item):
                    rrow, pb, mt, qsi = item
                    # gpsimd broadcast only works from base partition 0:
                    # fill all 128 partitions, use the [pb:pb+64] slice
                    prep_sb = pmisc.tile([128, 1024], BF16, tag="prep")
                    nc.gpsimd.partition_broadcast(prep_sb[:], rrow[:])
                    nc.vector.tensor_tensor(
                        y_norm_qs[qsi][pb : pb + 64, mt, :],
                        y_norm_qs[qsi][pb : pb + 64, mt, :],
                        prep_sb[pb : pb + 64, :],
                        MUL,
                    )

                def _emit_outproj(qsi, qhs=(0, 1)):
                    paired = len(qhs) == 2
                    for n in range(8):
                        ost2 = (
                            post.tile(
                                [128, 1024], BF16, tag="ost2", name="ost2"
                            )
                            if paired
                            else None
                        )
                        for qh in qhs:
                            qoff = 512 * qh
                            qi = 2 * qsi + qh
                            po = pso.tile([128, 512], F32, tag="out")
                            for c in range(2):
                                nc.tensor.matmul(
                                    po[:],
                                    wo[:, c, 128 * n : 128 * n + 128],
                                    y_norm_qs[qsi][:, c, qoff : qoff + 512],
                                    start=(c == 0),
                                    stop=(c == 1),
                                )
                            if paired:
                                dst = ost2[:, 512 * qh : 512 * qh + 512]
                            else:
                                dst = post.tile(
                                    [128, 512], BF16, tag="ost", name="ost"
                                )
                            if qsi == 1 and (n + qh) % 2 == 1:
                                nc.scalar.copy(dst, po[:])
                            else:
                                nc.vector.tensor_copy(dst, po[:])
                            if not paired:
                                [nc.sync, nc.scalar][n % 2].dma_start(
                                    outT[
                                        128 * n : 128 * n + 128,
                                        512 * qi : 512 * qi + 512,
                                    ],
                                    dst,
                                )
                        if paired:
                            # 2KB dst rows: one DMA per 128x1024 pair
                            [nc.sync, nc.scalar][n % 2].dma_start(
                                outT[
                                    128 * n : 128 * n + 128,
                                    1024 * qsi : 1024 * qsi + 1024,
                                ],
                                ost2[:],
                            )

                pending_norm = None
                for qs in range(2):
                    for h in range(HPC):
                        band = band_t[h]
                        mt = h // 2
                        q0 = 1024 * qs
                        n_live = 8 * (qs + 1)
                        live_half = [
                            min(4 * (2 * qs + j + 1), 16) for j in (0, 1)
                        ]
                        yT = psy.tile([HD + 1, 1024], F32, tag="yT")
                        last = (qs, h) == (1, HPC - 1)
                        if last:
                            # flush the pending norm now so its gpsimd
                            # broadcast can't wedge between this section's
                            # two split-chain broadcasts
                            _emit_norm(pending_norm)
                            pending_norm = None
                        pending = None
                        for ki in range(n_live):
                            # per-half spec: (j, s=trim start, const block?)
                            specs = []
                            for j in (0, 1):
                                if ki >= live_half[j]:
                                    continue
                                gq = q0 + 512 * j
                                s = max(0, 128 * ki - gq)
                                const_b = 128 * ki <= gq - 240
                                specs.append((j, s, const_b))
                            sp = psc.tile([128, 1024], F32, tag="score")
                            for j, s, const_b in specs:
                                gq = q0 + 512 * j
                                if not const_b:
                                    u0 = gq + s - 128 * ki
                                    nc.tensor.matmul(
                                        sp[:, 512 * j + s : 512 * j + 512],
                                        ident[:],
                                        band[:, u0 : u0 + 512 - s],
                                        start=True,
                                        stop=False,
                                    )
                                nc.tensor.matmul(
                                    sp[:, 512 * j + s : 512 * j + 512],
                                    kTt[mt][:, 128 * ki : 128 * ki + 128],
                                    qTz[h][:, gq + s : gq + 512],
                                    start=const_b,
                                    stop=True,
                                )
                            es = pes.tile([128, 1024], BF16, tag="es")
                            # bias is identically 0 (band carries the whole
                            # shifted rel-bias): one contiguous exp per ki
                            j0, s0, _ = specs[0]
                            lo = 512 * j0 + s0
                            hi = 512 * specs[-1][0] + 512
                            nc.scalar.activation(
                                es[:, lo:hi], sp[:, lo:hi], Exp
                            )
                            if pending is not None:
                                pes_t, pspecs, pki = pending
                                for j, s, _ in pspecs:
                                    nc.tensor.matmul(
                                        yT[:, 512 * j + s : 512 * j + 512],
                                        vxg[pki // 4][:, pki % 4, h, :],
                                        pes_t[:, 512 * j + s : 512 * j + 512],
                                        start=(pki == 0),
                                        stop=(pki == live_half[j] - 1),
                                    )
                                if False and pki == live_half[0] - 1:
                                    # j=0 half of the final section is done:
                                    # evacuate + normalize it now so the
                                    # out-projection can start immediately
                                    # after the last AV
                                    pbl = 64 * (h % 2)
                                    nc.vector.tensor_copy(
                                        y_norm_qs[qs][
                                            pbl : pbl + 64, h // 2, 0:512
                                        ],
                                        yT[0:HD, 0:512],
                                    )
                                    nc.vector.tensor_copy(
                                        dcp_l[:, 0:512],
                                        yT[HD : HD + 1, 0:512],
                                    )
                                    nc.sync.dma_start(
                                        dT_l[0:64, :], dcp_l[:, 0:512]
                                    )
                                    with nc.allow_low_precision(
                                        reason="softmax recip bf16"
                                    ):
                                        nc.vector.reciprocal(
                                            rT_l[0:64, :], dT_l[0:64, :]
                                        )
                                    nc.sync.dma_start(
                                        rrow_l[:, 0:512], rT_l[0:64, :]
                                    )
                                    nc.gpsimd.partition_broadcast(
                                        prep_l[:, 0:512], rrow_l[:, 0:512]
                                    )
                                    nc.vector.tensor_tensor(
                                        y_norm_qs[qs][
                                            pbl : pbl + 64, h // 2, 0:512
                                        ],
                                        y_norm_qs[qs][
                                            pbl : pbl + 64, h // 2, 0:512
                                        ],
                                        prep_l[pbl : pbl + 64, 0:512],
                                        MUL,
                                    )
                            pending = (es, specs, ki)
                        pes_t, pspecs, pki = pending
                        for j, s, _ in pspecs:
                            nc.tensor.matmul(
                                yT[:, 512 * j + s : 512 * j + 512],
                                vxg[pki // 4][:, pki % 4, h, :],
                                pes_t[:, 512 * j + s : 512 * j + 512],
                                start=(pki == 0),
                                stop=(pki == live_half[j] - 1),
                            )
                        pb = 64 * (h % 2)
                        if not last:
                            # yT evac FIRST (releases the single yT buffer
                            # for the next section ASAP), then the recip
                            # chain; the replication + in-place multiply for
                            # the PREVIOUS section is emitted now (its rrow
                            # is long ready), so nothing stalls on it.
                            nc.vector.tensor_copy(
                                y_norm_qs[qs][pb : pb + 64, mt, :],
                                yT[0:HD, :],
                            )
                            dcp = pmisc.tile([1, 1024], F32, tag="dcp")
                            nc.vector.tensor_copy(
                                dcp[:], yT[HD : HD + 1, :]
                            )
                            dT = pmisc.tile([128, 8], F32, tag="dT")
                            nc.sync.dma_start(dT[:], dcp[:])
                            rT = pmisc.tile([128, 8], BF16, tag="rT")
                            with nc.allow_low_precision(
                                reason="softmax recip bf16"
                            ):
                                nc.vector.reciprocal(rT[:], dT[:])
                            rrow = pmisc.tile([1, 1024], BF16, tag="rrow")
                            nc.sync.dma_start(rrow[:], rT[:])
                            if pending_norm is not None:
                                _emit_norm(pending_norm)
                            pending_norm = (rrow, pb, mt, qs)
                        else:
                            # final section: lean chain — one custom-DVE
                            # reciprocal straight off the PSUM denominator
                            # row, then fp32 PE replication (psum from the
                            # outproj pool). Fewer semaphore hops than the
                            # reshape/broadcast path.
                            dcp = pmisc.tile([1, 1024], F32, tag="dcp")
                            nc.vector.tensor_copy(
                                dcp[:], yT[HD : HD + 1, :]
                            )
                            rrec = pmisc.tile([1, 1024], F32, tag="rrec")
                            nc.vector.reciprocal_approx_fast(rrec[:], dcp[:])
                            nc.vector.tensor_copy(
                                y_norm_qs[qs][pb : pb + 64, mt, :],
                                yT[0:HD, :],
                            )
                            _emit_outproj(0, (1,))
                            for half in range(2):
                                hof = 512 * half
                                rep = pso.tile([64, 512], F32, tag="out")
                                nc.tensor.matmul(
                                    rep[:],
                                    ones_f[:],
                                    rrec[:, hof : hof + 512],
                                    start=True,
                                    stop=True,
                                )
                                nc.vector.tensor_tensor(
                                    y_norm_qs[qs][
                                        pb : pb + 64, mt, hof : hof + 512
                                    ],
                                    y_norm_qs[qs][
                                        pb : pb + 64, mt, hof : hof + 512
                                    ],
                                    rep[:],
                                    MUL,
                                )
                        if (qs, h) == (1, 0):
                            # norm for (3, qs=0) just got emitted: y_norm[0]
                            # is complete; its out-projection now overlaps
                            # the remaining qs=1 attention sections. qh=1
                            # stays back to fill the final-chain gap.
                            _emit_outproj(0, (0,))
                _emit_outproj(1)

    nc.finalize()
    return nc


def _host_tables(rel_emb: np.ndarray):
    """Per-core-group band tables; rel_emb is [NUM_BUCKETS, H]."""
    d = np.arange(4095)
    rp = d - 2047  # key - query
    buckets = _bucket(rp)
    bands = []
    c31s = []
    for h in range(H):
        # softmax is shift-invariant per row; subtracting the saturated
        # bucket-31 bias from the whole row makes the far-block exp bias
        # exactly 0 (no per-half bias bookkeeping on the ACT engine)
        c31 = np.float32(rel_emb[31, h])
        vals = rel_emb[buckets, h].astype(np.float32) - c31
        vals = np.where(rp > 0, np.float32(NEG), vals)  # causal mask
        band_pad = np.full(4223, NEG, np.float32)
        band_pad[:4095] = vals
        # BS[r, u] = band_pad[4095 + r - (2048 + u)]; reads only ever hit
        # x in [2048, 2816) of the full table
        idx = 2047 + np.arange(128)[:, None] - np.arange(768)[None, :]
        bands.append(band_pad[idx].astype(NP_BF16))
    return bands


def _numpy_ref(query, key, value, attn_mask, key_padding_mask,
               Wq, bq, Wk, bk, Wv, bv, Wo, bo, rel_emb):
    """Exact numpy fallback for unexpected mask patterns."""
    q = (query @ Wq.T + bq).reshape(B, L, H, HD).transpose(0, 2, 1, 3)
    k = (key @ Wk.T + bk).reshape(B, L, H, HD).transpose(0, 2, 1, 3)
    v = (value @ Wv.T + bv).reshape(B, L, H, HD).transpose(0, 2, 1, 3)
    scores = np.einsum("bhqd,bhkd->bhqk", q, k) / math.sqrt(HD)
    rp = np.arange(L, dtype=np.int64)[None, :] - np.arange(L, dtype=np.int64)[:, None]
    rel = rel_emb[_bucket(rp)].transpose(2, 0, 1)
    scores = scores + rel[None]
    scores = np.where(attn_mask[None, None], scores, -np.inf)
    scores = np.where(key_padding_mask[:, None, None, :], scores, -np.inf)
    scores = scores - scores.max(-1, keepdims=True)
    e = np.exp(scores)
    attn = e / e.sum(-1, keepdims=True)
    out = np.einsum("bhqk,bhkd->bhqd", attn, v)
    out = out.transpose(0, 2, 1, 3).reshape(B, L, D)
    return (out @ Wo.T + bo).astype(np.float32)


def kernel(**inputs) -> np.ndarray:
    global _cached, last_results
    inp = {k: np.asarray(v) for k, v in inputs.items()}
    query, key, value = inp["query"], inp["key"], inp["value"]
    attn_mask, kpm = inp["attn_mask"], inp["key_padding_mask"]
    Wq, bq, Wk, bk = inp["Wq"], inp["bq"], inp["Wk"], inp["bk"]
    Wv, bv, Wo, bo = inp["Wv"], inp["bv"], inp["Wo"], inp["bo"]
    rel_emb = inp["rel_emb"]

    causal = np.array_equal(attn_mask, np.tril(np.ones((L, L), bool)))
    if not (causal and kpm.all()):
        return _numpy_ref(**inp)

    if _cached is None:
        _cached = _build()
    nc = _cached

    bands = _host_tables(rel_emb)
    ident = np.eye(128, dtype=NP_BF16)

    def _rearr_w(w_slice):  # [MPC, D] row-major weights -> [128, 8, MPC]
        arr = np.ascontiguousarray(w_slice.T)  # [D, MPC]
        return arr.reshape(8, 128, MPC).transpose(1, 0, 2).astype(NP_BF16)

    def _rearr_w8(w_slice):
        arr = np.ascontiguousarray(w_slice.T)
        return arr.reshape(8, 128, MPC).transpose(1, 0, 2).astype(NP_FP8)

    in_maps = []
    for c in range(N_CORES):
        b, hg = c // HPC, c % HPC
        rows = slice(MPC * hg, MPC * hg + MPC)
        heads = range(HPC * hg, HPC * hg + HPC)
        wo_c = np.ascontiguousarray(Wo[:, rows].T)  # [MPC, D]
        in_maps.append({
            "qT_in": query[b].T.astype(NP_BF16),
            "kT_in": key[b].T.astype(NP_BF16),
            "vT_in": value[b].T.astype(NP_BF16),
            "wq_in": _rearr_w(Wq[rows] / math.sqrt(HD)),
            "wk_in": _rearr_w(Wk[rows]),
            "wv_in": _rearr_w(Wv[rows]),
            "wo_in": wo_c.reshape(2, 128, D).transpose(1, 0, 2).astype(NP_BF16),
            "bq_in": np.ascontiguousarray(
                (bq[rows] / math.sqrt(HD)).reshape(2, 128).T.astype(np.float32)
            ),
            "bk_in": np.ascontiguousarray(
                bk[rows].reshape(2, 128).T.astype(np.float32)
            ),
            "band_in": np.stack([bands[h] for h in heads]),
            "id_in": ident,
        })

    res = run_bass_kernel_spmd(nc, in_maps, list(range(N_CORES)))
    last_results = res

    bo_eff = (
        bo.astype(np.float64) + bv.astype(np.float64) @ Wo.T.astype(np.float64)
    )
    out = np.empty((B, L, D), np.float32)
    for b in range(B):
        acc = np.zeros((D, L), np.float64)
        for hg in range(HPC):
            acc += res.results[b * HPC + hg]["outT"].astype(np.float64)
        out[b] = (acc.T + bo_eff[None, :]).astype(np.float32)
    return out
